# revision 19
# baseline (speedup 1.0000x reference)
"""Differential attention (BiomedCLIP ViT-B) Bass kernel for 8 Trainium2 cores.

Strategy
--------
Data-parallel over batch: B=128 -> 16 batches per core, no collectives.

Host-side preprocessing folds the "differential" part away entirely:
  - lambda scalar lv is computed on host from lq1/lk1/lq2/lk2 (tiny tensors)
  - out = concat([x1 - lv*x2, x2]) @ Wp^T  ==  concat([x1, x2]) @ Wp_eff^T
    with Wp_eff = [Wp[:, :384], Wp[:, 384:] - lv*Wp[:, :384]]
  - attention scale 1/sqrt(hd) is folded into Wq/bq
so the device kernel is a standard 12-head MHA block.

Device-side layout: everything stays in "transposed" (feature-on-partition)
layout so no PE transposes are needed:
  Q^T = WqT.T @ xT           [o, t]   (o on partitions)
  K^T likewise               [o, t]
  V   = xT.T @ WvT (+ ones)  [t, o]   (natural; 65th column of ones per head)
  S^T = K_h @ Q_h^T          [tk, tq] per (batch, head)
  P^T = exp(S^T)             bf16
  U'^T = V'_h.T @ P^T        [65, tq] (row 64 = softmax denominators)
  O^T = U'^T[0:64] * (1/denom broadcast)
  Y   = O^T.T @ WpT_eff + bp [t, e]

Perf notes vs the first working version (641us -> 512us):
  - all-DVE softmax reciprocal: magic-constant seed bits(0x7EF311C3 - bits(d))
    via a uint32 tensor_sub on bitcast APs, plus one Newton step
    (d*r0 - 2)*r0 = -1/d (sign folded into host-negated WpT). The previous
    ACT-ReciprocAL seed alternated ACT table sets with Exp = 160 table loads
    = ~246us of scalar-engine time and serialization stalls; the scalar
    engine is now exp/Identity-only -> exactly ONE table load per run.
  - V'-ones memsets run on the otherwise-idle gpsimd engine, off the DVE.

Perf notes v2 (512us -> target ~330us):
  The trace showed DVE busy ~280us (vs PE warm work ~283us) and HAM
  throttle active 184us: PE kept stalling on the fp32 PSUM-source
  normalization chain (1x DVE mode, (120+FD)/0.96 ns per op) and went
  cold. Changes:
  - AV PSUM tile is copied once to SBUF as bf16 by the scalar engine
    (Identity); frees the PSUM bank immediately and the whole reciprocal
    chain + final muls run bf16 SBUF->SBUF in 2x_1P DVE mode (~half
    cost). Chain error (numpy-simulated): ~3.5e-3 rms on 1/d -- fine
    vs the 2e-2 gate.
  - bf16 198-column padding so every h01 block is 4B-aligned (2x mode
    requires it).
  - V'-ones columns persist across pairs (8 resident tiles, memset once).
  - startup: wq/bq on the sync DMA queue, pair-0 xT prefetch runs
    concurrently on the gpsimd queue; wk/wv/wp follow. Cuts the 20us
    cold-start PE stall to ~4us. xT prefetch stays on gpsimd queue.
"""

import sys
import os

sys.path.insert(0, "/opt/trn_rl_repo")

import numpy as np
import ml_dtypes

BF16 = ml_dtypes.bfloat16

# Problem constants (hardcoded per contract)
B, N, C = 128, 197, 768
NH, HD = 12, 64
NCORES = 8
BPC = B // NCORES          # batches per core = 16
T = BPC * N                # tokens per core = 3152
KC = C // 128              # contraction chunks = 6
NPAIR = NH // 2            # head pairs = 6
VW = NH * (HD + 1)         # V' width with ones columns = 780

_NC_CACHE = {}
LAST_RESULT = None         # BassKernelResults of the most recent run (for test.py)


def _build_nc(stage=4):
    """Build the Bass/Tile program (identical SPMD program for all 8 cores)."""
    import concourse.bass as bass
    from concourse import bacc, mybir
    from concourse.tile import TileContext
    from contextlib import ExitStack

    f32 = mybir.dt.float32
    bf16 = mybir.dt.bfloat16
    AF = mybir.ActivationFunctionType

    nc = bacc.Bacc(trn_type="TRN2", target_bir_lowering=False, debug=False)

    xT_d = nc.declare_dram_parameter("xT", [C, T], bf16, isOutput=False)
    wq_d = nc.declare_dram_parameter("wqT", [C, C], bf16, isOutput=False)
    wk_d = nc.declare_dram_parameter("wkT", [C, C], bf16, isOutput=False)
    wv_d = nc.declare_dram_parameter("wvT", [C, C], bf16, isOutput=False)
    wp_d = nc.declare_dram_parameter("wpT", [C, C], bf16, isOutput=False)
    bq_d = nc.declare_dram_parameter("bq2", [128, KC], f32, isOutput=False)
    bk_d = nc.declare_dram_parameter("bk2", [128, KC], f32, isOutput=False)
    bv_d = nc.declare_dram_parameter("bv", [C], f32, isOutput=False)
    bp_d = nc.declare_dram_parameter("bp", [C], f32, isOutput=False)
    y_d = nc.declare_dram_parameter("y", [T, C], f32, isOutput=True)

    PADN = N + 1  # 198: bf16 h01-block stride, keeps col offsets 4B-aligned

    with TileContext(nc) as tc, ExitStack() as ctx:
        consts = ctx.enter_context(tc.tile_pool(name="consts", bufs=1))
        xt_pool = ctx.enter_context(tc.tile_pool(name="xt", bufs=2))
        qk_pool = ctx.enter_context(tc.tile_pool(name="qk", bufs=2))
        vper = ctx.enter_context(tc.tile_pool(name="vper", bufs=1))
        ut_pool = ctx.enter_context(tc.tile_pool(name="ut", bufs=2))
        p_pool = ctx.enter_context(tc.tile_pool(name="pp", bufs=6))
        r_pool = ctx.enter_context(tc.tile_pool(name="rr", bufs=4))
        y_pool = ctx.enter_context(tc.tile_pool(name="yy", bufs=3))
        psum = ctx.enter_context(tc.tile_pool(name="psum", bufs=2, space="PSUM"))

        # --- constants; DMA order matters for the cold start: the first QK
        # matmuls need wq + pair-0 xT only. wq/bq go first on the sync
        # queue while pair-0 xT streams concurrently on the gpsimd queue;
        # wk..wp follow (they are needed progressively later).
        def load_w(d, nm, engs):
            lst = []
            for k in range(KC):
                t_ = consts.tile([128, C], bf16, tag=f"{nm}{k}")
                engs[k % len(engs)].dma_start(
                    out=t_, in_=d[k * 128:(k + 1) * 128, :])
                lst.append(t_)
            return lst

        wq = load_w(wq_d, "wq", [nc.sync])
        bq2 = consts.tile([128, KC], f32, tag="bq2")
        nc.sync.dma_start(out=bq2, in_=bq_d[:, :])

        def emit_xt(p):
            t0 = p * (2 * N)
            lst = []
            for k in range(KC):
                t_ = xt_pool.tile([128, 2 * N], bf16, tag=f"xt{k}")
                nc.gpsimd.dma_start(
                    out=t_, in_=xT_d[k * 128:(k + 1) * 128, t0:t0 + 2 * N])
                lst.append(t_)
            return lst

        xt_cur = emit_xt(0)

        wk = load_w(wk_d, "wk", [nc.sync])
        bk2 = consts.tile([128, KC], f32, tag="bk2")
        nc.sync.dma_start(out=bk2, in_=bk_d[:, :])
        # NOTE: no DMAs on the scalar queue -- DMA_DIRECT2D blocks the ACT
        # engine for the whole transfer (~590ns each), which starves the
        # prologue QK Identity reads and stalls the PE ~10us. wv/wp/biases
        # ride the otherwise-idle gpsimd queue in need-order instead.
        wv = load_w(wv_d, "wv", [nc.gpsimd])
        bvb = consts.tile([128, C], f32, tag="bvb")
        bv_ap = bv_d[:]
        nc.gpsimd.dma_start(
            out=bvb, in_=bass.AP(tensor=bv_ap.tensor, offset=0, ap=[[0, 128], [1, C]])
        )
        wp = load_w(wp_d, "wp", [nc.gpsimd])
        bpb = consts.tile([128, C], f32, tag="bpb")
        bp_ap = bp_d[:]
        nc.gpsimd.dma_start(
            out=bpb, in_=bass.AP(tensor=bp_ap.tensor, offset=0, ap=[[0, 128], [1, C]])
        )
        zeros = consts.tile([128, 1], f32, tag="zeros")
        nc.vector.memset(zeros, 0.0)
        # constant tile for the fast-reciprocal bit trick, bf16 flavour:
        # bits16(r0) = 0x7EF3 - bits16(d), computed as a uint16 tensor_sub
        u16 = mybir.dt.uint16
        magic16 = consts.tile([128, 2 * PADN], u16, tag="magic16")
        nc.vector.memset(magic16, 0x7EF3)

        # persistent V'' tiles: 8 resident buffers (b01, ts, pair-parity).
        # Odd 64-col half of each head block is all-ones (folds the softmax
        # denominator into the AV matmul); V-proj only ever rewrites the
        # even halves, so the ones survive across pairs -> memset ONCE.
        vtiles = {}
        for b01 in range(2):
            for ts in range(2):
                for par in range(2):
                    vt = vper.tile([128, NH * 128], bf16, tag=f"v{b01}{ts}{par}")
                    vt3 = vt.rearrange("p (h c) -> p h c", c=128)
                    nc.gpsimd.memset(vt3[:, :, HD:128], 1.0)
                    vtiles[(b01, ts, par)] = vt

        BANK = 512  # fp32 elements per PSUM bank

        # --- pipelined stream ---
        # The attention S/AV sequence is the "spine"; every other matmul
        # group (QK proj, V proj, Y proj, each ~1us of PE work) is a filler
        # unit drained from a pending queue between spine ops. This keeps
        # the PE dense while the ACT-exp -> DVE-reciprocal chain recycles
        # the AV PSUM banks, and keeps HAM at K=8/8.
        # Steady state, pair p:
        #   attn(p,b0)  (x)  fillers [Y(p-1,b1), V(p,b1)]
        #   attn(p,b1)  (x)  fillers [Y(p,b0), QK(p+1), V(p+1,b0)]
        NP = BPC // 2
        pending = []

        def fill(n=1):
            for _ in range(min(n, len(pending))):
                pending.pop(0)()

        def qk_units(xt, qt, kt):
            units = []
            for wts, bias_t, out_list, nm in ((wq, bq2, qt, "qt"),
                                              (wk, bk2, kt, "kt")):
                for j in range(KC):
                    def emit(wts=wts, bias_t=bias_t, out_list=out_list,
                             nm=nm, j=j):
                        ps = psum.tile([128, 2 * N], f32, tag="proj")
                        for k in range(KC):
                            nc.tensor.matmul(
                                ps, lhsT=wts[k][:, j * 128:(j + 1) * 128],
                                rhs=xt[k],
                                start=(k == 0), stop=(k == KC - 1),
                            )
                        sb = qk_pool.tile([128, 2 * N], bf16, tag=f"{nm}{j}")
                        nc.scalar.activation(sb, ps, AF.Identity,
                                             bias=bias_t[:, j:j + 1])
                        out_list.append(sb)
                    units.append(emit)
            return units

        def v_units(p, b01, xt):
            units = []
            for ts in range(2):
                for oh in range(2):
                    def emit(ts=ts, oh=oh):
                        tsz = 128 if ts == 0 else N - 128
                        vt = vtiles[(b01, ts, p % 2)]
                        vt3 = vt.rearrange("p (h c) -> p h c", c=128)
                        tc0 = b01 * N + ts * 128
                        ps = psum.tile([128, 384], f32, tag="proj")
                        for k in range(KC):
                            nc.tensor.matmul(
                                ps[:tsz],
                                lhsT=xt[k][:, tc0:tc0 + tsz],
                                rhs=wv[k][:, oh * 384:(oh + 1) * 384],
                                start=(k == 0), stop=(k == KC - 1),
                            )
                        nc.vector.tensor_add(
                            out=vt3[:tsz, oh * 6:(oh + 1) * 6, 0:HD],
                            in0=ps[:tsz].rearrange("p (h d) -> p h d", d=HD),
                            in1=bvb[:tsz].rearrange("p (h d) -> p h d", d=HD)[
                                :, oh * 6:(oh + 1) * 6, :],
                        )
                    units.append(emit)
            return units

        def y_units(p, b01, ut):
            units = []
            ysbs = {}
            for ts in range(2):
                for eh in range(2):
                    def emit(ts=ts, eh=eh):
                        tsz = 128 if ts == 0 else N - 128
                        trow = (2 * p + b01) * N + ts * 128
                        if eh == 0:
                            ysbs[ts] = y_pool.tile([128, C], f32, tag="y",
                                                   name=f"ysb_{p}_{b01}_{ts}")
                        ysb = ysbs[ts]
                        ps = psum.tile([128, 384], f32, tag="proj")
                        for j in range(KC):
                            nc.tensor.matmul(
                                ps[:tsz],
                                lhsT=ut[j][:, ts * 128:ts * 128 + tsz],
                                rhs=wp[j][:, eh * 384:(eh + 1) * 384],
                                start=(j == 0), stop=(j == KC - 1),
                            )
                        nc.vector.tensor_add(
                            out=ysb[:tsz, eh * 384:(eh + 1) * 384],
                            in0=ps[:tsz],
                            in1=bpb[:tsz, eh * 384:(eh + 1) * 384],
                        )
                        if eh == 1:
                            nc.sync.dma_start(out=y_d[trow:trow + tsz, :],
                                              in_=ysb[:tsz])
                    units.append(emit)
            return units

        def attn_batch(p, b01, qt, kt, ut):
            bcol = b01 * N
            pts = {}

            def emit_s(hp):
                for tkc in range(2):
                    tksz = 128 if tkc == 0 else N - 128
                    tks0 = bcol + tkc * 128
                    # two banks in one tile: head even -> bank 0, head odd
                    # (weights at partition base 64, row group 1) -> bank 1
                    sps = psum.tile([128, 2 * BANK], f32, tag="s")
                    for h01 in range(2):
                        nc.tensor.matmul(
                            sps[:tksz, h01 * BANK:h01 * BANK + N],
                            lhsT=kt[hp][h01 * HD:(h01 + 1) * HD,
                                        tks0:tks0 + tksz],
                            rhs=qt[hp][h01 * HD:(h01 + 1) * HD,
                                       bcol:bcol + N],
                            start=True, stop=True,
                        )
                    pt = p_pool.tile([128, 2 * PADN], bf16, tag="pt")
                    nc.scalar.activation(
                        pt[:tksz].rearrange("p (two x) -> p two x",
                                            x=PADN)[:, :, 0:N],
                        sps[:tksz].rearrange("p (two x) -> p two x",
                                             x=BANK)[:, :, 0:N],
                        AF.Exp, bias=zeros[:tksz])
                    pts[(hp, tkc)] = pt

            def emit_av(hp):
                ups = psum.tile([128, 2 * N], f32, tag="u")
                nmm = 0
                for h01 in range(2):
                    h = hp * 2 + h01
                    for tkc in range(2):
                        tksz = 128 if tkc == 0 else N - 128
                        nc.tensor.matmul(
                            ups[:, h01 * N:(h01 + 1) * N],
                            lhsT=vtiles[(b01, tkc, p % 2)][
                                :tksz, h * 128:(h + 1) * 128],
                            rhs=pts[(hp, tkc)][:tksz,
                                               h01 * PADN:h01 * PADN + N],
                            start=(nmm == 0), stop=(nmm == 3),
                        )
                        nmm += 1
                # ACT Identity copies the replicated denominators d (rows
                # 64:128) to SBUF bf16; the reciprocal chain runs bf16
                # SBUF->SBUF in 2x DVE mode: magic seed
                # bits16(0x7EF3 - bits16(d)) + one Newton step
                # (d*r0 - 2)*r0 = -1/d (sign folded into host-negated WpT).
                # 198-col block stride keeps 4B alignment; all SB operands
                # sit on partitions 64:128 (BIR: SBUF tensor-tensor inputs
                # share the start partition); final muls read U from PSUM.
                usb = r_pool.tile([128, 2 * PADN], bf16, tag="usb")
                nc.scalar.activation(
                    usb.rearrange("p (two x) -> p two x",
                                  x=PADN)[HD:128, :, 0:N],
                    ups.rearrange("p (two x) -> p two x",
                                  x=N)[HD:128, :, :],
                    AF.Identity, bias=zeros[HD:128])
                r0t = r_pool.tile([128, 2 * PADN], bf16, tag="r0")
                nc.vector.tensor_sub(
                    out=r0t[HD:128, :].bitcast(u16),
                    in0=magic16[HD:128, :],
                    in1=usb[HD:128, :].bitcast(u16))
                tt = r_pool.tile([128, 2 * PADN], bf16, tag="tt")
                nc.vector.tensor_mul(out=tt[HD:128, :],
                                     in0=usb[HD:128, :],
                                     in1=r0t[HD:128, :])
                rsb = r_pool.tile([128, 2 * PADN], bf16, tag="rsb")
                nc.vector.scalar_tensor_tensor(
                    out=rsb[HD:128, :], in0=tt[HD:128, :], scalar=2.0,
                    in1=r0t[HD:128, :],
                    op0=mybir.AluOpType.subtract,
                    op1=mybir.AluOpType.mult)
                for h01 in range(2):
                    r0 = h01 * HD
                    nc.vector.tensor_mul(
                        out=ut[hp][r0:r0 + HD, :],
                        in0=ups[0:HD, h01 * N:h01 * N + N],
                        in1=rsb[HD:128, h01 * PADN:h01 * PADN + N],
                    )

            emit_s(0)
            fill()
            for hp in range(1, NPAIR):
                emit_s(hp)
                fill()
                emit_av(hp - 1)
                fill(2)
            emit_av(NPAIR - 1)
            fill(2)

        xts = {0: xt_cur}
        qts = {0: []}
        kts = {0: []}
        uts = {}

        for p in range(NP):
            if p + 1 < NP:
                xts[p + 1] = emit_xt(p + 1)
                xts.pop(p - 1, None)
            if p == 0:
                # prologue: nothing to overlap with yet -- emit as blocks
                for u in qk_units(xts[0], qts[0], kts[0]):
                    u()
                for u in v_units(0, 0, xts[0]):
                    u()
            fill(len(pending))  # leftover units from iter p-1
            if p > 0:
                pending.extend(y_units(p - 1, 1, uts[(p - 1, 1)]))
            pending.extend(v_units(p, 1, xts[p]))
            if p + 1 < NP:
                qts[p + 1] = []
                kts[p + 1] = []
                pending.extend(qk_units(xts[p + 1], qts[p + 1], kts[p + 1]))
                pending.extend(v_units(p + 1, 0, xts[p + 1]))
            uts[(p, 0)] = [ut_pool.tile([128, N], bf16, tag=f"ut{j}",
                                        name=f"ut{j}_{p}_0")
                           for j in range(KC)]
            attn_batch(p, 0, qts[p], kts[p], uts[(p, 0)])
            pending.extend(y_units(p, 0, uts[(p, 0)]))
            uts[(p, 1)] = [ut_pool.tile([128, N], bf16, tag=f"ut{j}",
                                        name=f"ut{j}_{p}_1")
                           for j in range(KC)]
            attn_batch(p, 1, qts[p], kts[p], uts[(p, 1)])

        fill(len(pending))
        for u in y_units(NP - 1, 1, uts[(NP - 1, 1)]):
            u()

    nc.finalize()
    return nc


def _prep_inputs(x, Wq, bq, Wk, bk, Wv, bv, Wp, bp, lq1, lk1, lq2, lk2,
                 lambda_init):
    """Host-side preprocessing -> per-core input maps."""
    f32 = np.float32
    x = np.asarray(x, f32)
    Wq = np.asarray(Wq, f32); bq = np.asarray(bq, f32)
    Wk = np.asarray(Wk, f32); bk = np.asarray(bk, f32)
    Wv = np.asarray(Wv, f32); bv = np.asarray(bv, f32)
    Wp = np.asarray(Wp, f32); bp = np.asarray(bp, f32)

    # lambda scalar (float32 math like the jax reference)
    l1 = np.exp(np.minimum((np.asarray(lq1, f32) * np.asarray(lk1, f32)).sum(axis=(-1, -2)), f32(5.0)))
    l2 = np.exp(np.minimum((np.asarray(lq2, f32) * np.asarray(lk2, f32)).sum(axis=(-1, -2)), f32(5.0)))
    lv = f32((l1 - l2 + f32(lambda_init)).mean())

    scale = f32(HD ** -0.5)
    wqT = np.ascontiguousarray((Wq * scale).T).astype(BF16)
    wkT = np.ascontiguousarray(Wk.T).astype(BF16)
    wvT = np.ascontiguousarray(Wv.T).astype(BF16)
    Wp_eff = np.concatenate([Wp[:, :C // 2], Wp[:, C // 2:] - lv * Wp[:, :C // 2]],
                            axis=1)
    # negated: the device normalization computes -1/denom (see kernel)
    wpT = np.ascontiguousarray(-Wp_eff.T).astype(BF16)

    bq2 = np.ascontiguousarray((bq * scale).reshape(KC, 128).T).astype(f32)
    bk2 = np.ascontiguousarray(bk.reshape(KC, 128).T).astype(f32)

    in_maps = []
    for c in range(NCORES):
        xc = x[c * BPC:(c + 1) * BPC].reshape(T, C)
        xT = np.ascontiguousarray(xc.T).astype(BF16)
        in_maps.append({
            "xT": xT, "wqT": wqT, "wkT": wkT, "wvT": wvT, "wpT": wpT,
            "bq2": bq2, "bk2": bk2, "bv": bv, "bp": bp,
        })
    return in_maps


def kernel(x, Wq, bq, Wk, bk, Wv, bv, Wp, bp, lq1, lk1, lq2, lk2,
           num_heads, lambda_init):
    global LAST_RESULT
    from concourse.bass_utils import run_bass_kernel_spmd

    assert int(num_heads) == NH
    assert tuple(np.asarray(x).shape) == (B, N, C)

    if "nc" not in _NC_CACHE:
        _NC_CACHE["nc"] = _build_nc()
    nc = _NC_CACHE["nc"]

    in_maps = _prep_inputs(x, Wq, bq, Wk, bk, Wv, bv, Wp, bp,
                           lq1, lk1, lq2, lk2, lambda_init)
    res = run_bass_kernel_spmd(nc, in_maps, list(range(NCORES)))
    LAST_RESULT = res
    out = np.concatenate(
        [res.results[c]["y"].reshape(BPC, N, C) for c in range(NCORES)], axis=0
    )
    return np.ascontiguousarray(out.astype(np.float32))



# revision 21
# speedup vs baseline: 1.2013x; 1.2013x over previous
"""Differential attention (BiomedCLIP ViT-B) Bass kernel for 8 Trainium2 cores.

Strategy
--------
Data-parallel over batch: B=128 -> 16 batches per core, no collectives.

Host-side preprocessing folds the "differential" part away entirely:
  - lambda scalar lv is computed on host from lq1/lk1/lq2/lk2 (tiny tensors)
  - out = concat([x1 - lv*x2, x2]) @ Wp^T  ==  concat([x1, x2]) @ Wp_eff^T
    with Wp_eff = [Wp[:, :384], Wp[:, 384:] - lv*Wp[:, :384]]
  - attention scale 1/sqrt(hd) is folded into Wq/bq
so the device kernel is a standard 12-head MHA block.

Device-side layout: everything stays in "transposed" (feature-on-partition)
layout so no PE transposes are needed:
  Q^T = WqT.T @ xT           [o, t]   (o on partitions)
  K^T likewise               [o, t]
  V   = xT.T @ WvT (+ ones)  [t, o]   (natural; 65th column of ones per head)
  S^T = K_h @ Q_h^T          [tk, tq] per (batch, head)
  P^T = exp(S^T)             bf16
  U'^T = V'_h.T @ P^T        [65, tq] (row 64 = softmax denominators)
  O^T = U'^T[0:64] * (1/denom broadcast)
  Y   = O^T.T @ WpT_eff + bp [t, e]

Perf notes vs the first working version (641us -> 512us):
  - all-DVE softmax reciprocal: magic-constant seed bits(0x7EF311C3 - bits(d))
    via a uint32 tensor_sub on bitcast APs, plus one Newton step
    (d*r0 - 2)*r0 = -1/d (sign folded into host-negated WpT). The previous
    ACT-ReciprocAL seed alternated ACT table sets with Exp = 160 table loads
    = ~246us of scalar-engine time and serialization stalls; the scalar
    engine is now exp/Identity-only -> exactly ONE table load per run.
  - V'-ones memsets run on the otherwise-idle gpsimd engine, off the DVE.

Perf notes v2 (512us -> target ~330us):
  The trace showed DVE busy ~280us (vs PE warm work ~283us) and HAM
  throttle active 184us: PE kept stalling on the fp32 PSUM-source
  normalization chain (1x DVE mode, (120+FD)/0.96 ns per op) and went
  cold. Changes:
  - AV PSUM tile is copied once to SBUF as bf16 by the scalar engine
    (Identity); frees the PSUM bank immediately and the whole reciprocal
    chain + final muls run bf16 SBUF->SBUF in 2x_1P DVE mode (~half
    cost). Chain error (numpy-simulated): ~3.5e-3 rms on 1/d -- fine
    vs the 2e-2 gate.
  - bf16 198-column padding so every h01 block is 4B-aligned (2x mode
    requires it).
  - V'-ones columns persist across pairs (8 resident tiles, memset once).
  - startup: wq/bq on the sync DMA queue, pair-0 xT prefetch runs
    concurrently on the gpsimd queue; wk/wv/wp follow. Cuts the 20us
    cold-start PE stall to ~4us. xT prefetch stays on gpsimd queue.
"""

import sys
import os

sys.path.insert(0, "/opt/trn_rl_repo")

import numpy as np
import ml_dtypes

BF16 = ml_dtypes.bfloat16

# Problem constants (hardcoded per contract)
B, N, C = 128, 197, 768
NH, HD = 12, 64
NCORES = 8
BPC = B // NCORES          # batches per core = 16
T = BPC * N                # tokens per core = 3152
KC = C // 128              # contraction chunks = 6
NPAIR = NH // 2            # head pairs = 6
VW = NH * (HD + 1)         # V' width with ones columns = 780

_NC_CACHE = {}
LAST_RESULT = None         # BassKernelResults of the most recent run (for test.py)


def _build_nc(stage=4):
    """Build the Bass/Tile program (identical SPMD program for all 8 cores)."""
    import concourse.bass as bass
    from concourse import bacc, mybir
    from concourse.tile import TileContext
    from contextlib import ExitStack

    f32 = mybir.dt.float32
    bf16 = mybir.dt.bfloat16
    AF = mybir.ActivationFunctionType

    nc = bacc.Bacc(trn_type="TRN2", target_bir_lowering=False, debug=False)

    xT_d = nc.declare_dram_parameter("xT", [C, T], bf16, isOutput=False)
    wq_d = nc.declare_dram_parameter("wqT", [C, C], bf16, isOutput=False)
    wk_d = nc.declare_dram_parameter("wkT", [C, C], bf16, isOutput=False)
    wv_d = nc.declare_dram_parameter("wvT", [C, C], bf16, isOutput=False)
    wp_d = nc.declare_dram_parameter("wpT", [C, C], bf16, isOutput=False)
    bq_d = nc.declare_dram_parameter("bq2", [128, KC], f32, isOutput=False)
    bk_d = nc.declare_dram_parameter("bk2", [128, KC], f32, isOutput=False)
    bv_d = nc.declare_dram_parameter("bv", [C], f32, isOutput=False)
    bp_d = nc.declare_dram_parameter("bp", [C], f32, isOutput=False)
    y_d = nc.declare_dram_parameter("y", [T, C], f32, isOutput=True)

    PADN = N + 1  # 198: bf16 h01-block stride, keeps col offsets 4B-aligned

    with TileContext(nc) as tc, ExitStack() as ctx:
        consts = ctx.enter_context(tc.tile_pool(name="consts", bufs=1))
        xt_pool = ctx.enter_context(tc.tile_pool(name="xt", bufs=2))
        qk_pool = ctx.enter_context(tc.tile_pool(name="qk", bufs=2))
        vper = ctx.enter_context(tc.tile_pool(name="vper", bufs=1))
        ut_pool = ctx.enter_context(tc.tile_pool(name="ut", bufs=2))
        p_pool = ctx.enter_context(tc.tile_pool(name="pp", bufs=6))
        r_pool = ctx.enter_context(tc.tile_pool(name="rr", bufs=4))
        y_pool = ctx.enter_context(tc.tile_pool(name="yy", bufs=3))
        psum = ctx.enter_context(tc.tile_pool(name="psum", bufs=2, space="PSUM"))

        # --- constants; DMA order matters for the cold start: the first QK
        # matmuls need wq + pair-0 xT only. wq/bq go first on the sync
        # queue while pair-0 xT streams concurrently on the gpsimd queue;
        # wk..wp follow (they are needed progressively later).
        def load_w(d, nm, engs):
            lst = []
            for k in range(KC):
                t_ = consts.tile([128, C], bf16, tag=f"{nm}{k}")
                engs[k % len(engs)].dma_start(
                    out=t_, in_=d[k * 128:(k + 1) * 128, :])
                lst.append(t_)
            return lst

        wq = load_w(wq_d, "wq", [nc.sync])
        bq2 = consts.tile([128, KC], f32, tag="bq2")
        nc.sync.dma_start(out=bq2, in_=bq_d[:, :])

        def emit_xt(p):
            t0 = p * (2 * N)
            lst = []
            for k in range(KC):
                t_ = xt_pool.tile([128, 2 * N], bf16, tag=f"xt{k}")
                nc.gpsimd.dma_start(
                    out=t_, in_=xT_d[k * 128:(k + 1) * 128, t0:t0 + 2 * N])
                lst.append(t_)
            return lst

        xt_cur = emit_xt(0)

        wk = load_w(wk_d, "wk", [nc.sync])
        bk2 = consts.tile([128, KC], f32, tag="bk2")
        nc.sync.dma_start(out=bk2, in_=bk_d[:, :])
        wv = load_w(wv_d, "wv", [nc.sync])
        wp = load_w(wp_d, "wp", [nc.sync])
        bvb = consts.tile([128, C], f32, tag="bvb")
        bv_ap = bv_d[:]
        nc.sync.dma_start(
            out=bvb, in_=bass.AP(tensor=bv_ap.tensor, offset=0, ap=[[0, 128], [1, C]])
        )
        bpb = consts.tile([128, C], f32, tag="bpb")
        bp_ap = bp_d[:]
        nc.sync.dma_start(
            out=bpb, in_=bass.AP(tensor=bp_ap.tensor, offset=0, ap=[[0, 128], [1, C]])
        )
        zeros = consts.tile([128, 1], f32, tag="zeros")
        nc.vector.memset(zeros, 0.0)
        # constant tile for the fast-reciprocal bit trick, bf16 flavour:
        # bits16(r0) = 0x7EF3 - bits16(d), computed as a uint16 tensor_sub
        u16 = mybir.dt.uint16
        magic16 = consts.tile([128, 2 * PADN], u16, tag="magic16")
        nc.vector.memset(magic16, 0x7EF3)

        # persistent V'' tiles: 8 resident buffers (b01, ts, pair-parity).
        # Odd 64-col half of each head block is all-ones (folds the softmax
        # denominator into the AV matmul); V-proj only ever rewrites the
        # even halves, so the ones survive across pairs -> memset ONCE.
        vtiles = {}
        for b01 in range(2):
            for ts in range(2):
                for par in range(2):
                    vt = vper.tile([128, NH * 128], bf16, tag=f"v{b01}{ts}{par}")
                    vt3 = vt.rearrange("p (h c) -> p h c", c=128)
                    nc.gpsimd.memset(vt3[:, :, HD:128], 1.0)
                    vtiles[(b01, ts, par)] = vt

        BANK = 512  # fp32 elements per PSUM bank

        # --- pipelined stream ---
        # The attention S/AV sequence is the "spine"; every other matmul
        # group (QK proj, V proj, Y proj, each ~1us of PE work) is a filler
        # unit drained from a pending queue between spine ops. This keeps
        # the PE dense while the ACT-exp -> DVE-reciprocal chain recycles
        # the AV PSUM banks, and keeps HAM at K=8/8.
        # Steady state, pair p:
        #   attn(p,b0)  (x)  fillers [Y(p-1,b1), V(p,b1)]
        #   attn(p,b1)  (x)  fillers [Y(p,b0), QK(p+1), V(p+1,b0)]
        NP = BPC // 2
        pending = []

        def fill(n=1):
            for _ in range(min(n, len(pending))):
                pending.pop(0)()

        def qk_units(xt, qt, kt):
            units = []
            for wts, bias_t, out_list, nm in ((wq, bq2, qt, "qt"),
                                              (wk, bk2, kt, "kt")):
                for j in range(KC):
                    def emit(wts=wts, bias_t=bias_t, out_list=out_list,
                             nm=nm, j=j):
                        ps = psum.tile([128, 2 * N], f32, tag="proj")
                        for k in range(KC):
                            nc.tensor.matmul(
                                ps, lhsT=wts[k][:, j * 128:(j + 1) * 128],
                                rhs=xt[k],
                                start=(k == 0), stop=(k == KC - 1),
                            )
                        sb = qk_pool.tile([128, 2 * N], bf16, tag=f"{nm}{j}")
                        nc.scalar.activation(sb, ps, AF.Identity,
                                             bias=bias_t[:, j:j + 1])
                        out_list.append(sb)
                    units.append(emit)
            return units

        def v_units(p, b01, xt):
            units = []
            for ts in range(2):
                for oh in range(2):
                    def emit(ts=ts, oh=oh):
                        tsz = 128 if ts == 0 else N - 128
                        vt = vtiles[(b01, ts, p % 2)]
                        vt3 = vt.rearrange("p (h c) -> p h c", c=128)
                        tc0 = b01 * N + ts * 128
                        ps = psum.tile([128, 384], f32, tag="proj")
                        for k in range(KC):
                            nc.tensor.matmul(
                                ps[:tsz],
                                lhsT=xt[k][:, tc0:tc0 + tsz],
                                rhs=wv[k][:, oh * 384:(oh + 1) * 384],
                                start=(k == 0), stop=(k == KC - 1),
                            )
                        nc.vector.tensor_add(
                            out=vt3[:tsz, oh * 6:(oh + 1) * 6, 0:HD],
                            in0=ps[:tsz].rearrange("p (h d) -> p h d", d=HD),
                            in1=bvb[:tsz].rearrange("p (h d) -> p h d", d=HD)[
                                :, oh * 6:(oh + 1) * 6, :],
                        )
                    units.append(emit)
            return units

        def y_units(p, b01, ut):
            units = []
            ysbs = {}
            for ts in range(2):
                for eh in range(2):
                    def emit(ts=ts, eh=eh):
                        tsz = 128 if ts == 0 else N - 128
                        trow = (2 * p + b01) * N + ts * 128
                        if eh == 0:
                            ysbs[ts] = y_pool.tile([128, C], f32, tag="y",
                                                   name=f"ysb_{p}_{b01}_{ts}")
                        ysb = ysbs[ts]
                        ps = psum.tile([128, 384], f32, tag="proj")
                        for j in range(KC):
                            nc.tensor.matmul(
                                ps[:tsz],
                                lhsT=ut[j][:, ts * 128:ts * 128 + tsz],
                                rhs=wp[j][:, eh * 384:(eh + 1) * 384],
                                start=(j == 0), stop=(j == KC - 1),
                            )
                        nc.vector.tensor_add(
                            out=ysb[:tsz, eh * 384:(eh + 1) * 384],
                            in0=ps[:tsz],
                            in1=bpb[:tsz, eh * 384:(eh + 1) * 384],
                        )
                        if eh == 1:
                            nc.sync.dma_start(out=y_d[trow:trow + tsz, :],
                                              in_=ysb[:tsz])
                    units.append(emit)
            return units

        def attn_batch(p, b01, qt, kt, ut):
            bcol = b01 * N
            pts = {}

            def emit_s(hp):
                for tkc in range(2):
                    tksz = 128 if tkc == 0 else N - 128
                    tks0 = bcol + tkc * 128
                    # two banks in one tile: head even -> bank 0, head odd
                    # (weights at partition base 64, row group 1) -> bank 1
                    sps = psum.tile([128, 2 * BANK], f32, tag="s")
                    for h01 in range(2):
                        nc.tensor.matmul(
                            sps[:tksz, h01 * BANK:h01 * BANK + N],
                            lhsT=kt[hp][h01 * HD:(h01 + 1) * HD,
                                        tks0:tks0 + tksz],
                            rhs=qt[hp][h01 * HD:(h01 + 1) * HD,
                                       bcol:bcol + N],
                            start=True, stop=True,
                        )
                    pt = p_pool.tile([128, 2 * PADN], bf16, tag="pt")
                    nc.scalar.activation(
                        pt[:tksz].rearrange("p (two x) -> p two x",
                                            x=PADN)[:, :, 0:N],
                        sps[:tksz].rearrange("p (two x) -> p two x",
                                             x=BANK)[:, :, 0:N],
                        AF.Exp, bias=zeros[:tksz])
                    pts[(hp, tkc)] = pt

            def emit_av(hp):
                ups = psum.tile([128, 2 * N], f32, tag="u")
                nmm = 0
                for h01 in range(2):
                    h = hp * 2 + h01
                    for tkc in range(2):
                        tksz = 128 if tkc == 0 else N - 128
                        nc.tensor.matmul(
                            ups[:, h01 * N:(h01 + 1) * N],
                            lhsT=vtiles[(b01, tkc, p % 2)][
                                :tksz, h * 128:(h + 1) * 128],
                            rhs=pts[(hp, tkc)][:tksz,
                                               h01 * PADN:h01 * PADN + N],
                            start=(nmm == 0), stop=(nmm == 3),
                        )
                        nmm += 1
                # ACT Identity copies the replicated denominators d (rows
                # 64:128) to SBUF bf16; the reciprocal chain runs bf16
                # SBUF->SBUF in 2x DVE mode: magic seed
                # bits16(0x7EF3 - bits16(d)) + one Newton step
                # (d*r0 - 2)*r0 = -1/d (sign folded into host-negated WpT).
                # 198-col block stride keeps 4B alignment; all SB operands
                # sit on partitions 64:128 (BIR: SBUF tensor-tensor inputs
                # share the start partition); final muls read U from PSUM.
                usb = r_pool.tile([128, 2 * PADN], bf16, tag="usb")
                nc.scalar.activation(
                    usb.rearrange("p (two x) -> p two x",
                                  x=PADN)[HD:128, :, 0:N],
                    ups.rearrange("p (two x) -> p two x",
                                  x=N)[HD:128, :, :],
                    AF.Identity, bias=zeros[HD:128])
                r0t = r_pool.tile([128, 2 * PADN], bf16, tag="r0")
                nc.vector.tensor_sub(
                    out=r0t[HD:128, :].bitcast(u16),
                    in0=magic16[HD:128, :],
                    in1=usb[HD:128, :].bitcast(u16))
                tt = r_pool.tile([128, 2 * PADN], bf16, tag="tt")
                nc.vector.tensor_mul(out=tt[HD:128, :],
                                     in0=usb[HD:128, :],
                                     in1=r0t[HD:128, :])
                rsb = r_pool.tile([128, 2 * PADN], bf16, tag="rsb")
                nc.vector.scalar_tensor_tensor(
                    out=rsb[HD:128, :], in0=tt[HD:128, :], scalar=2.0,
                    in1=r0t[HD:128, :],
                    op0=mybir.AluOpType.subtract,
                    op1=mybir.AluOpType.mult)
                for h01 in range(2):
                    r0 = h01 * HD
                    nc.vector.tensor_mul(
                        out=ut[hp][r0:r0 + HD, :],
                        in0=ups[0:HD, h01 * N:h01 * N + N],
                        in1=rsb[HD:128, h01 * PADN:h01 * PADN + N],
                    )

            emit_s(0)
            fill()
            for hp in range(1, NPAIR):
                emit_s(hp)
                fill()
                emit_av(hp - 1)
                fill(2)
            emit_av(NPAIR - 1)
            fill(2)

        xts = {0: xt_cur}
        qts = {0: []}
        kts = {0: []}
        uts = {}

        for p in range(NP):
            if p + 1 < NP:
                xts[p + 1] = emit_xt(p + 1)
                xts.pop(p - 1, None)
            if p == 0:
                # prologue: nothing to overlap with yet -- emit as blocks
                for u in qk_units(xts[0], qts[0], kts[0]):
                    u()
                for u in v_units(0, 0, xts[0]):
                    u()
            fill(len(pending))  # leftover units from iter p-1
            if p > 0:
                pending.extend(y_units(p - 1, 1, uts[(p - 1, 1)]))
            pending.extend(v_units(p, 1, xts[p]))
            if p + 1 < NP:
                qts[p + 1] = []
                kts[p + 1] = []
                pending.extend(qk_units(xts[p + 1], qts[p + 1], kts[p + 1]))
                pending.extend(v_units(p + 1, 0, xts[p + 1]))
            uts[(p, 0)] = [ut_pool.tile([128, N], bf16, tag=f"ut{j}",
                                        name=f"ut{j}_{p}_0")
                           for j in range(KC)]
            attn_batch(p, 0, qts[p], kts[p], uts[(p, 0)])
            pending.extend(y_units(p, 0, uts[(p, 0)]))
            uts[(p, 1)] = [ut_pool.tile([128, N], bf16, tag=f"ut{j}",
                                        name=f"ut{j}_{p}_1")
                           for j in range(KC)]
            attn_batch(p, 1, qts[p], kts[p], uts[(p, 1)])

        fill(len(pending))
        for u in y_units(NP - 1, 1, uts[(NP - 1, 1)]):
            u()

    nc.finalize()
    return nc


def _prep_inputs(x, Wq, bq, Wk, bk, Wv, bv, Wp, bp, lq1, lk1, lq2, lk2,
                 lambda_init):
    """Host-side preprocessing -> per-core input maps."""
    f32 = np.float32
    x = np.asarray(x, f32)
    Wq = np.asarray(Wq, f32); bq = np.asarray(bq, f32)
    Wk = np.asarray(Wk, f32); bk = np.asarray(bk, f32)
    Wv = np.asarray(Wv, f32); bv = np.asarray(bv, f32)
    Wp = np.asarray(Wp, f32); bp = np.asarray(bp, f32)

    # lambda scalar (float32 math like the jax reference)
    l1 = np.exp(np.minimum((np.asarray(lq1, f32) * np.asarray(lk1, f32)).sum(axis=(-1, -2)), f32(5.0)))
    l2 = np.exp(np.minimum((np.asarray(lq2, f32) * np.asarray(lk2, f32)).sum(axis=(-1, -2)), f32(5.0)))
    lv = f32((l1 - l2 + f32(lambda_init)).mean())

    scale = f32(HD ** -0.5)
    wqT = np.ascontiguousarray((Wq * scale).T).astype(BF16)
    wkT = np.ascontiguousarray(Wk.T).astype(BF16)
    wvT = np.ascontiguousarray(Wv.T).astype(BF16)
    Wp_eff = np.concatenate([Wp[:, :C // 2], Wp[:, C // 2:] - lv * Wp[:, :C // 2]],
                            axis=1)
    # negated: the device normalization computes -1/denom (see kernel)
    wpT = np.ascontiguousarray(-Wp_eff.T).astype(BF16)

    bq2 = np.ascontiguousarray((bq * scale).reshape(KC, 128).T).astype(f32)
    bk2 = np.ascontiguousarray(bk.reshape(KC, 128).T).astype(f32)

    in_maps = []
    for c in range(NCORES):
        xc = x[c * BPC:(c + 1) * BPC].reshape(T, C)
        xT = np.ascontiguousarray(xc.T).astype(BF16)
        in_maps.append({
            "xT": xT, "wqT": wqT, "wkT": wkT, "wvT": wvT, "wpT": wpT,
            "bq2": bq2, "bk2": bk2, "bv": bv, "bp": bp,
        })
    return in_maps


def kernel(x, Wq, bq, Wk, bk, Wv, bv, Wp, bp, lq1, lk1, lq2, lk2,
           num_heads, lambda_init):
    global LAST_RESULT
    from concourse.bass_utils import run_bass_kernel_spmd

    assert int(num_heads) == NH
    assert tuple(np.asarray(x).shape) == (B, N, C)

    if "nc" not in _NC_CACHE:
        _NC_CACHE["nc"] = _build_nc()
    nc = _NC_CACHE["nc"]

    in_maps = _prep_inputs(x, Wq, bq, Wk, bk, Wv, bv, Wp, bp,
                           lq1, lk1, lq2, lk2, lambda_init)
    res = run_bass_kernel_spmd(nc, in_maps, list(range(NCORES)))
    LAST_RESULT = res
    out = np.concatenate(
        [res.results[c]["y"].reshape(BPC, N, C) for c in range(NCORES)], axis=0
    )
    return np.ascontiguousarray(out.astype(np.float32))

